# revision 1
# baseline (speedup 1.0000x reference)
"""BatchCenterLoss Trainium2 kernel (8 NeuronCores, SPMD via bass_utils).

Loss = sum over same-class pairs (i != j) of ||x_i - x_j|| / 2 / B.

Strategy -- class-sharded data-parallel: only same-class pairs contribute,
so instead of the full 16384^2 distance matrix (268M entries) the host
computes class-sort indices (the sharding step), each core indirect-DMA
gathers its 13 class blocks (padded to C=256 rows) on device, and computes
only the 104 block-diagonal CxC distance tiles (~6.8M entries, ~40x less
work). Per block b:
  - gather C rows -> nat chunks; PE-transpose into xgT [D=128, C]
  - row norms n via PE ones-matmuls over sqb = xb*xb ([1,C] row vector for
    the column term, [128,1] per row-tile for the Relu bias; -1e9 pad
    penalties folded in with one small DVE add each)
  - PSUM: g - 0.5*(n_c + q_c) from a K=128 matmul + K=1 accumulate matmul
  - ACT Relu(scale=-2, bias=n_r + q_r) -> t1 = relu(||xi-xj||^2 + q terms)
    (padded slots see ~-1e9 and die here; any gather value works for pads)
  - DVE multiply diagonal subtile by (1-I) to kill i==j
  - ACT Sqrt with accum_out -> per-row sums rs[:, tile]
rs [128, 26] is DMA'd out per core; the host sums (float64) and scales by
1/(2B).

Hardware notes (learned the hard way; sim does NOT catch these):
  - indirect_dma_start offsets must be [128, 1]: multi-offset gathers pass
    CoreSim but return garbage on TRN2.
  - build on bacc.Bacc and call nc.compile() -- it splits multi-semaphore
    waits that walrus's LDWEIGHTS lowering cannot encode.
  - engines cannot address SBUF starting at partition 1 (only 0/32/64/96);
    SBUF->SBUF DMA can, if ever needed.
"""

from contextlib import ExitStack

import numpy as np

import concourse.bass as bass
import concourse.tile as tile
from concourse import bacc, mybir
from concourse.bass_utils import run_bass_kernel_spmd
from concourse.masks import make_identity

B = 16384
D = 128
NCLS = 100
NCORES = 8
NBLK = 13

F32 = mybir.dt.float32
I32 = mybir.dt.int32

_prog_cache = {}
TRACE = False
LAST_RESULTS = None


def _build(C, iters=1):
    R = NBLK * C
    CH = R // 128
    CPB = C // 128

    nc = bacc.Bacc("TRN2", target_bir_lowering=False, debug=False)
    xa = nc.dram_tensor("xa", [B, D], F32, kind="ExternalInput").ap()
    idx = nc.dram_tensor("idx", [128, CH], I32, kind="ExternalInput").ap()
    qrow = nc.dram_tensor("qrow", [1, R], F32, kind="ExternalInput").ap()
    pcol = nc.dram_tensor("pcol", [128, CH], F32, kind="ExternalInput").ap()
    out = nc.dram_tensor("out", [128, CH], F32, kind="ExternalOutput").ap()

    with ExitStack() as ctx:
        tc = ctx.enter_context(tile.TileContext(nc))
        const = ctx.enter_context(tc.tile_pool(name="const", bufs=1))
        natp = ctx.enter_context(tc.tile_pool(name="nat", bufs=4))
        sqbp = ctx.enter_context(tc.tile_pool(name="sqb", bufs=2))
        nbp = ctx.enter_context(tc.tile_pool(name="nb", bufs=3))
        t1p = ctx.enter_context(tc.tile_pool(name="t1", bufs=3))
        t2p = ctx.enter_context(tc.tile_pool(name="t2", bufs=2))
        pstp = ctx.enter_context(tc.tile_pool(name="pst", bufs=2, space="PSUM"))
        psgp = ctx.enter_context(tc.tile_pool(name="psg", bufs=4, space="PSUM"))
        # one pool, two tags: psn [1,C] + nbp [128,1]; bufs=1 keeps PSUM <= 8 banks
        psnp = ctx.enter_context(tc.tile_pool(name="psn", bufs=1, space="PSUM"))

        identity = const.tile([128, 128], F32)
        make_identity(nc, identity[:])
        notI = const.tile([128, 128], F32)
        nc.gpsimd.memset(notI[:], 1.0)
        nc.gpsimd.affine_select(
            out=notI[:],
            in_=notI[:],
            compare_op=mybir.AluOpType.not_equal,
            fill=0.0,
            base=0,
            pattern=[[-1, 128]],
            channel_multiplier=1,
        )
        ones_col = const.tile([128, 1], F32)
        nc.vector.memset(ones_col[:], 1.0)
        neghalf = const.tile([1, 128], F32)
        nc.vector.memset(neghalf[:], -0.5)

        idx_sb = const.tile([128, CH], I32)
        nc.sync.dma_start(out=idx_sb[:], in_=idx)
        qrow_sb = const.tile([1, R], F32)
        nc.sync.dma_start(out=qrow_sb[:], in_=qrow)
        pcol_sb = const.tile([128, CH], F32)
        nc.sync.dma_start(out=pcol_sb[:], in_=pcol)

        xgT = const.tile([128, R], F32)
        rs = const.tile([128, CH], F32)

        for b in [bb for _ in range(iters) for bb in range(NBLK)]:
            for cc in range(CPB):
                c = b * CPB + cc
                nat = natp.tile([128, 128], F32)
                nc.gpsimd.indirect_dma_start(
                    out=nat[:],
                    out_offset=None,
                    in_=xa[:, :],
                    in_offset=bass.IndirectOffsetOnAxis(ap=idx_sb[:, c : c + 1], axis=0),
                )
                pst = pstp.tile([128, 128], F32)
                nc.tensor.transpose(out=pst[:], in_=nat[:], identity=identity[:])
                nc.vector.tensor_copy(out=xgT[:, c * 128 : (c + 1) * 128], in_=pst[:])
            xb = xgT[:, b * C : (b + 1) * C]
            sqb = sqbp.tile([128, C], F32)
            nc.vector.tensor_tensor(
                out=sqb[:], in0=xb, in1=xb, op=mybir.AluOpType.mult
            )
            psn = psnp.tile([1, C], F32, tag="psn")
            nc.tensor.matmul(out=psn[:], lhsT=ones_col[:], rhs=sqb[:], start=True, stop=True)
            nb_row = nbp.tile([1, C], F32, tag="nb_row")
            nc.vector.tensor_add(
                out=nb_row[:], in0=psn[:], in1=qrow_sb[:, b * C : (b + 1) * C]
            )
            for h in range(CPB):
                r = b * CPB + h
                # row norms for the Relu bias: PE ones-matmul over sqb slice,
                # then one DVE add folds in the pad penalty (replaces the ACT
                # Square pass -- ACT is the bottleneck engine)
                nbp_ps = psnp.tile([128, 1], F32, tag="nbp")
                nc.tensor.matmul(
                    out=nbp_ps[:],
                    lhsT=sqb[:, h * 128 : (h + 1) * 128],
                    rhs=ones_col[:],
                    start=True,
                    stop=True,
                )
                nb_aug = nbp.tile([128, 1], F32, tag="nb_aug")
                nc.vector.tensor_add(
                    out=nb_aug[:],
                    in0=nbp_ps[:],
                    in1=pcol_sb[:, r : r + 1],
                )
                psg = psgp.tile([128, C], F32)
                nc.tensor.matmul(
                    out=psg[:],
                    lhsT=xgT[:, r * 128 : (r + 1) * 128],
                    rhs=xb,
                    start=True,
                    stop=False,
                )
                nc.tensor.matmul(
                    out=psg[:], lhsT=neghalf[:], rhs=nb_row[:], start=False, stop=True
                )
                t1 = t1p.tile([128, C], F32)
                nc.scalar.activation(
                    out=t1[:],
                    in_=psg[:],
                    func=mybir.ActivationFunctionType.Relu,
                    bias=nb_aug[:, 0:1],
                    scale=-2.0,
                )
                nc.vector.tensor_tensor(
                    out=t1[:, h * 128 : (h + 1) * 128],
                    in0=t1[:, h * 128 : (h + 1) * 128],
                    in1=notI[:],
                    op=mybir.AluOpType.mult,
                )
                t2 = t2p.tile([128, C], F32)
                nc.scalar.activation(
                    out=t2[:],
                    in_=t1[:],
                    func=mybir.ActivationFunctionType.Sqrt,
                    accum_out=rs[:, r : r + 1],
                )

        nc.sync.dma_start(out=out[:, :], in_=rs[:])

    nc.compile()
    return nc


def _prep_inputs(x, target, C):
    R = NBLK * C
    CH = R // 128
    t = np.asarray(target).astype(np.int64).ravel()
    order = np.argsort(t, kind="stable").astype(np.int32)
    counts = np.bincount(t, minlength=NCORES * NBLK)
    starts = np.concatenate([[0], np.cumsum(counts)])

    xa = np.ascontiguousarray(np.asarray(x, dtype=np.float32))

    in_maps = []
    for core in range(NCORES):
        idx = np.zeros((R,), dtype=np.int32)  # pad -> row 0; penalties kill it
        pen = np.full((R,), -1e9, dtype=np.float32)
        for b in range(NBLK):
            k = core * NBLK + b
            cnt = int(counts[k]) if k < len(counts) else 0
            if cnt > 0:
                idx[b * C : b * C + cnt] = order[starts[k] : starts[k] + cnt]
                pen[b * C : b * C + cnt] = 0.0
        in_maps.append(
            {
                "xa": xa,
                "idx": np.ascontiguousarray(idx.reshape(CH, 128).T),
                "qrow": pen.reshape(1, R),
                "pcol": np.ascontiguousarray(pen.reshape(CH, 128).T),
            }
        )
    return in_maps


def kernel(x, target):
    t = np.asarray(target).astype(np.int64).ravel()
    counts = np.bincount(t, minlength=NCLS)
    C = max(256, ((int(counts.max()) + 127) // 128) * 128)
    if C not in _prog_cache:
        _prog_cache[C] = _build(C)
    nc = _prog_cache[C]
    in_maps = _prep_inputs(x, target, C)
    global LAST_RESULTS
    results = run_bass_kernel_spmd(nc, in_maps, list(range(NCORES)), trace=TRACE)
    LAST_RESULTS = results
    total = float(sum(np.asarray(r["out"], dtype=np.float64).sum() for r in results.results))
    return np.float32(total / 2.0 / B)



# revision 2
# speedup vs baseline: 2.5424x; 2.5424x over previous
"""BatchCenterLoss Trainium2 kernel (8 NeuronCores, SPMD via bass_utils).

Loss = sum over same-class pairs (i != j) of ||x_i - x_j|| / 2 / B.

Strategy v3 -- class-sharded data-parallel, single-ACT-pass:
Only same-class pairs contribute. The host sorts rows by class, assigns
classes to (core, slot) round-robin by descending count (slot b across all
cores holds classes of similar size), and uploads per core:
  - xgT  [128=D, R] bf16: the core's rows PRE-TRANSPOSED, each slot padded
    to 256 rows (pad cols are zero),
  - rowb [2, R]  bf16: p0 = 1/0 (real/pad), p1 = row norm n_i (0 for pad),
  - colb [2, RW] bf16: per-slot W-wide col windows: p0 = -(0.5 n_j + delta)
    (0 for pad), p1 = -0.5 (0 for pad).
Per row-tile (slot b, half h) the PE produces, in PSUM,
  psg = x_r^T x_c - 0.5 n_r - 0.5 n_c - delta          (= -0.5 sqdist - d)
with ONE K=128 gram matmul plus ONE K=2 rank-1 matmul (cost on PE is
K-independent). Pad rows/cols come out EXACTLY 0 (zero x cols and zero
rank-1 multipliers), so no Relu pass and no diagonal mask are needed: the
ACT engine does a single in-place Sqrt(scale=-2) with accum_out row-sums
over a 4-bank strided PSUM access pattern. delta keeps the diagonal's fp
noise inside Sqrt's valid domain [0, 2^118]; the host subtracts the
predicted diagonal contribution (known to fp32 accumulation-order noise)
and scales by 1/(2B).

Engine budget per core (cost model): ACT ~5.7us (bottleneck), PE ~4-7us,
DMA ~3us, DVE/Pool 0. Baseline v1 was ACT ~26us + Pool ~29us (indirect
DMA descriptor generation).

Hardware notes (learned the hard way; sim does NOT catch these):
  - build on bacc.Bacc and call nc.compile() -- it splits multi-semaphore
    waits that walrus's LDWEIGHTS lowering cannot encode.
  - engines cannot address SBUF starting at partition 1 (only 0/32/64/96).
"""

from contextlib import ExitStack

import numpy as np
import ml_dtypes

import concourse.bass as bass
import concourse.tile as tile
from concourse import bacc, mybir
from concourse.bass_utils import run_bass_kernel_spmd

B = 16384
D = 128
NCLS = 100
NCORES = 8
NSLOT = 13
SLOT_ROWS = 256
R = NSLOT * SLOT_ROWS
GRPSLOTS = ((0, 1, 2, 3), (4, 5, 6, 7), (8, 9, 10, 11), (12,))
NG = len(GRPSLOTS)
DELTA = 0.5

F32 = mybir.dt.float32
BF16 = mybir.dt.bfloat16
NP_BF16 = np.dtype(ml_dtypes.bfloat16)

_prog_cache = {}
TRACE = False
LAST_RESULTS = None


def _build(ws):
    """ws: per-slot col-window widths (uniform within each ACT group)."""
    offs = np.concatenate([[0], np.cumsum(ws)]).astype(int)
    RW = int(offs[-1])

    nc = bacc.Bacc("TRN2", target_bir_lowering=False, debug=False)
    xgt = nc.dram_tensor("xgt", [128, R], BF16, kind="ExternalInput").ap()
    rowb = nc.dram_tensor("rowb", [2, R], BF16, kind="ExternalInput").ap()
    colb = nc.dram_tensor("colb", [2, RW], BF16, kind="ExternalInput").ap()
    out = nc.dram_tensor("out", [128, NG], F32, kind="ExternalOutput").ap()

    with ExitStack() as ctx:
        tc = ctx.enter_context(tile.TileContext(nc))
        const = ctx.enter_context(tc.tile_pool(name="const", bufs=1))
        psp = ctx.enter_context(tc.tile_pool(name="ps", bufs=2, space="PSUM"))

        xgt_sb = const.tile([128, R], BF16)
        rowb_sb = const.tile([2, R], BF16)
        colb_sb = const.tile([2, RW], BF16)
        rs = const.tile([128, NG], F32)

        nc.sync.dma_start(out=rowb_sb[:], in_=rowb)
        nc.sync.dma_start(out=colb_sb[:], in_=colb)
        # chunked xgT loads: one per ACT group so PE starts after chunk 0
        for slots in GRPSLOTS:
            lo = slots[0] * SLOT_ROWS
            hi = (slots[-1] + 1) * SLOT_ROWS
            nc.sync.dma_start(out=xgt_sb[:, lo:hi], in_=xgt[:, lo:hi])

        for g, slots in enumerate(GRPSLOTS):
            W = int(ws[slots[0]])
            grp = psp.tile([128, 2048], F32, tag="grp")
            for qi, s in enumerate(slots):
                base = s * SLOT_ROWS
                O = int(offs[s])
                q0 = qi * 512
                for h in (0, 1):
                    dst = grp[:, q0 + h * W : q0 + (h + 1) * W]
                    nc.tensor.matmul(
                        out=dst,
                        lhsT=xgt_sb[:, base + h * 128 : base + h * 128 + 128],
                        rhs=xgt_sb[:, base : base + W],
                        start=True,
                        stop=False,
                    )
                    nc.tensor.matmul(
                        out=dst,
                        lhsT=rowb_sb[:, base + h * 128 : base + h * 128 + 128],
                        rhs=colb_sb[:, O : O + W],
                        start=False,
                        stop=True,
                    )
            if len(slots) > 1:
                ap_in = grp[:].rearrange("p (b w) -> p b w", w=512)[
                    :, 0 : len(slots), 0 : 2 * W
                ]
            else:
                ap_in = grp[:, 0 : 2 * W]
            nc.scalar.activation(
                out=ap_in,
                in_=ap_in,
                func=mybir.ActivationFunctionType.Sqrt,
                scale=-2.0,
                accum_out=rs[:, g : g + 1],
            )

        nc.sync.dma_start(out=out[:, :], in_=rs[:])

    nc.compile()
    return nc


def _plan(counts):
    """Assign class ranks to (slot, core); slot widths uniform per ACT group."""
    ranks = np.argsort(counts, kind="stable")[::-1]  # class ids, count desc
    ws = np.zeros(NSLOT, dtype=np.int64)
    for slots in GRPSLOTS:
        k0 = 8 * slots[0]
        mx = int(counts[ranks[k0]]) if k0 < len(ranks) else 8
        W = min(((mx + 7) // 8) * 8, SLOT_ROWS)
        for s in slots:
            ws[s] = W
    return ranks, ws


def _prep_inputs(x, target, ranks, ws):
    offs = np.concatenate([[0], np.cumsum(ws)]).astype(int)
    RW = int(offs[-1])
    t = np.asarray(target).astype(np.int64).ravel()
    order = np.argsort(t, kind="stable")
    counts = np.bincount(t, minlength=NCLS)
    starts = np.concatenate([[0], np.cumsum(counts)])

    xb16 = np.asarray(x, dtype=np.float32).astype(NP_BF16)
    xb32 = xb16.astype(np.float32)
    nrm = (xb32 * xb32).sum(axis=1, dtype=np.float32)  # matches PE's fp32 acc

    in_maps = []
    diag_corr = 0.0
    for core in range(NCORES):
        xgT = np.zeros((128, R), dtype=np.float32)
        rowb = np.zeros((2, R), dtype=np.float32)
        colb = np.zeros((2, RW), dtype=np.float32)
        for b in range(NSLOT):
            k = 8 * b + core
            if k >= NCLS:
                continue
            cls = int(ranks[k])
            cnt = int(counts[cls])
            rows = order[starts[cls] : starts[cls] + cnt]
            base = b * SLOT_ROWS
            O = int(offs[b])
            xgT[:, base : base + cnt] = xb32[rows].T
            rowb[0, base : base + cnt] = 1.0
            rowb[1, base : base + cnt] = nrm[rows]
            colb[0, O : O + cnt] = -(0.5 * nrm[rows] + DELTA)
            colb[1, O : O + cnt] = -0.5
            # predicted diagonal contribution (device computes bf16-rounded
            # rank-1 terms against the exact fp32 gram diagonal = nrm)
            c0 = colb[0, O : O + cnt].astype(NP_BF16).astype(np.float64)
            r1 = rowb[1, base : base + cnt].astype(NP_BF16).astype(np.float64)
            t1 = -2.0 * (nrm[rows].astype(np.float64) + c0 - 0.5 * r1)
            diag_corr += np.sqrt(np.maximum(t1, 0.0)).sum()
        in_maps.append(
            {
                "xgt": np.ascontiguousarray(xgT.astype(NP_BF16)),
                "rowb": np.ascontiguousarray(rowb.astype(NP_BF16)),
                "colb": np.ascontiguousarray(colb.astype(NP_BF16)),
            }
        )
    return in_maps, diag_corr


def kernel(x, target):
    t = np.asarray(target).astype(np.int64).ravel()
    counts = np.bincount(t, minlength=NCLS)
    assert counts.max() <= SLOT_ROWS, "class larger than a slot"
    ranks, ws = _plan(counts)
    key = tuple(int(w) for w in ws)
    if key not in _prog_cache:
        _prog_cache[key] = _build(ws)
    nc = _prog_cache[key]
    in_maps, diag_corr = _prep_inputs(x, target, ranks, ws)
    global LAST_RESULTS
    results = run_bass_kernel_spmd(nc, in_maps, list(range(NCORES)), trace=TRACE)
    LAST_RESULTS = results
    total = float(
        sum(np.asarray(r["out"], dtype=np.float64).sum() for r in results.results)
    )
    total -= diag_corr
    return np.float32(total / 2.0 / B)


# revision 6
# speedup vs baseline: 3.5900x; 1.4120x over previous
"""BatchCenterLoss Trainium2 kernel (8 NeuronCores, SPMD via bass_utils).

Loss = sum over same-class pairs (i != j) of ||x_i - x_j|| / 2 / B.

Strategy v3 -- class-sharded data-parallel, single-ACT-pass:
Only same-class pairs contribute. The host sorts rows by class, assigns
classes to (core, slot) round-robin by descending count (slot b across all
cores holds classes of similar size), and uploads per core:
  - xgT  [128=D, R] bf16: the core's rows PRE-TRANSPOSED, each slot padded
    to 256 rows (pad cols are zero),
  - rowb [2, R]  bf16: p0 = 1/0 (real/pad), p1 = row norm n_i (0 for pad),
  - colb [2, RW] bf16: per-slot W-wide col windows: p0 = -(0.5 n_j + delta)
    (0 for pad), p1 = -0.5 (0 for pad).
Per row-tile (slot b, half h) the PE produces, in PSUM,
  psg = x_r^T x_c - 0.5 n_r - 0.5 n_c - delta          (= -0.5 sqdist - d)
with ONE K=128 gram matmul plus ONE K=2 rank-1 matmul (cost on PE is
K-independent). Pad rows/cols come out EXACTLY 0 (zero x cols and zero
rank-1 multipliers), so no Relu pass and no diagonal mask are needed: the
ACT engine does a single in-place Sqrt(scale=-2) with accum_out row-sums
over a 4-bank strided PSUM access pattern. delta keeps the diagonal's fp
noise inside Sqrt's valid domain [0, 2^118]; the host subtracts the
predicted diagonal contribution (known to fp32 accumulation-order noise)
and scales by 1/(2B).

Engine budget per core (cost model): ACT ~5.7us (bottleneck), PE ~4-7us,
DMA ~3us, DVE/Pool 0. Baseline v1 was ACT ~26us + Pool ~29us (indirect
DMA descriptor generation).

Hardware notes (learned the hard way; sim does NOT catch these):
  - build on bacc.Bacc and call nc.compile() -- it splits multi-semaphore
    waits that walrus's LDWEIGHTS lowering cannot encode.
  - engines cannot address SBUF starting at partition 1 (only 0/32/64/96).
"""

from contextlib import ExitStack

import numpy as np
import ml_dtypes

import concourse.bass as bass
import concourse.tile as tile
from concourse import bacc, mybir
from concourse.bass_utils import run_bass_kernel_spmd

B = 16384
D = 128
NCLS = 100
NCORES = 8
NSLOT = 13
SLOT_ROWS = 256
R = NSLOT * SLOT_ROWS
GRPSLOTS = ((0, 1, 2, 3), (4, 5, 6, 7), (8, 9, 10, 11), (12,))
NG = len(GRPSLOTS)
DELTA = 0.5
NWARM = 16  # PE pstate warmup matmuls (first ~11 PE instrs run below peak)
# xgt DMA chunks (slot ranges) + issue order with the aux buffer: c0 first so
# slot-0 grams start ASAP, aux second (gates every rank-1), rest pipelined.
XCHUNKS = ((0, 4), (4, 8), (8, 13))

F32 = mybir.dt.float32
BF16 = mybir.dt.bfloat16
FP8 = mybir.dt.float8e4
NP_BF16 = np.dtype(ml_dtypes.bfloat16)
NP_FP8 = np.dtype(mybir.dt.np(FP8))

_prog_cache = {}
TRACE = False
LAST_RESULTS = None


def _build(ws):
    """ws: per-slot col-window widths (uniform within each ACT group)."""
    offs = np.concatenate([[0], np.cumsum(ws)]).astype(int)
    RW = int(offs[-1])

    nc = bacc.Bacc("TRN2", target_bir_lowering=False, debug=False)
    xgt = nc.dram_tensor("xgt", [128, R], FP8, kind="ExternalInput").ap()
    aux = nc.dram_tensor("aux", [2, R + RW], BF16, kind="ExternalInput").ap()
    out = nc.dram_tensor("out", [128, NG], F32, kind="ExternalOutput").ap()

    with ExitStack() as ctx:
        tc = ctx.enter_context(tile.TileContext(nc))
        const = ctx.enter_context(tc.tile_pool(name="const", bufs=1))
        psp = ctx.enter_context(tc.tile_pool(name="ps", bufs=2, space="PSUM"))

        xgt_sb = const.tile([128, R], FP8)
        aux_sb = const.tile([2, R + RW], BF16)
        rs = const.tile([128, NG], F32)
        wsb = const.tile([128, 16], BF16)

        # PE pstate warmup: engine idle until DMA lands; burn the slow-ramp
        # instructions on tiny matmuls whose results get overwritten.
        nc.vector.memset(wsb[:], 1.0)

        g_tiles = [
            psp.tile([128, 2048], F32, tag="grp", name=f"grp{g}") for g in range(NG)
        ]
        for i in range(NWARM):
            nc.tensor.matmul(
                out=g_tiles[0][0:16, 0:16],
                lhsT=wsb[:, 0:16],
                rhs=wsb[:, 0:16],
                start=True,
                stop=True,
            )

        # DMA order: first xgt chunk, then aux, then remaining chunks.
        lo, hi = XCHUNKS[0][0] * SLOT_ROWS, XCHUNKS[0][1] * SLOT_ROWS
        nc.sync.dma_start(out=xgt_sb[:, lo:hi], in_=xgt[:, lo:hi])
        nc.sync.dma_start(out=aux_sb[:], in_=aux)
        for s0, s1 in XCHUNKS[1:]:
            lo, hi = s0 * SLOT_ROWS, s1 * SLOT_ROWS
            nc.sync.dma_start(out=xgt_sb[:, lo:hi], in_=xgt[:, lo:hi])

        for g, slots in enumerate(GRPSLOTS):
            W = int(ws[slots[0]])
            grp = g_tiles[g]
            for qi, s in enumerate(slots):
                base = s * SLOT_ROWS
                O = int(offs[s])
                q0 = qi * 512
                for h in (0, 1):
                    dst = grp[:, q0 + h * W : q0 + (h + 1) * W]
                    nc.tensor.matmul(
                        out=dst,
                        lhsT=xgt_sb[:, base + h * 128 : base + h * 128 + 128],
                        rhs=xgt_sb[:, base : base + W],
                        start=True,
                        stop=False,
                    )
                    nc.tensor.matmul(
                        out=dst,
                        lhsT=aux_sb[:, base + h * 128 : base + h * 128 + 128],
                        rhs=aux_sb[:, R + O : R + O + W],
                        start=False,
                        stop=True,
                    )
            if len(slots) > 1:
                ap_in = grp[:].rearrange("p (b w) -> p b w", w=512)[
                    :, 0 : len(slots), 0 : 2 * W
                ]
            else:
                ap_in = grp[:, 0 : 2 * W]
            nc.scalar.activation(
                out=ap_in,
                in_=ap_in,
                func=mybir.ActivationFunctionType.Sqrt,
                scale=-2.0,
                accum_out=rs[:, g : g + 1],
            )

        nc.sync.dma_start(out=out[:, :], in_=rs[:])

    nc.compile()
    return nc


def _plan(counts):
    """Assign class ranks to (slot, core); slot widths uniform per ACT group."""
    ranks = np.argsort(counts, kind="stable")[::-1]  # class ids, count desc
    ws = np.zeros(NSLOT, dtype=np.int64)
    for slots in GRPSLOTS:
        k0 = 8 * slots[0]
        mx = int(counts[ranks[k0]]) if k0 < len(ranks) else 8
        W = min(((mx + 7) // 8) * 8, SLOT_ROWS)
        for s in slots:
            ws[s] = W
    return ranks, ws


def _prep_inputs(x, target, ranks, ws):
    offs = np.concatenate([[0], np.cumsum(ws)]).astype(int)
    RW = int(offs[-1])
    t = np.asarray(target).astype(np.int64).ravel()
    order = np.argsort(t, kind="stable")
    counts = np.bincount(t, minlength=NCLS)
    starts = np.concatenate([[0], np.cumsum(counts)])

    x8 = np.asarray(x, dtype=np.float32).astype(NP_FP8)
    x832 = x8.astype(np.float32)
    nrm = (x832 * x832).sum(axis=1, dtype=np.float32)  # matches PE's fp32 acc

    in_maps = []
    diag_corr = 0.0
    for core in range(NCORES):
        xgT = np.zeros((128, R), dtype=np.float32)
        auxb = np.zeros((2, R + RW), dtype=np.float32)
        for b in range(NSLOT):
            k = 8 * b + core
            if k >= NCLS:
                continue
            cls = int(ranks[k])
            cnt = int(counts[cls])
            rows = order[starts[cls] : starts[cls] + cnt]
            base = b * SLOT_ROWS
            O = int(offs[b])
            xgT[:, base : base + cnt] = x832[rows].T
            auxb[0, base : base + cnt] = 1.0
            auxb[1, base : base + cnt] = nrm[rows]
            auxb[0, R + O : R + O + cnt] = -(0.5 * nrm[rows] + DELTA)
            auxb[1, R + O : R + O + cnt] = -0.5
            # predicted diagonal contribution (device computes bf16-rounded
            # rank-1 terms against the exact fp32 gram diagonal = nrm)
            c0 = auxb[0, R + O : R + O + cnt].astype(NP_BF16).astype(np.float64)
            r1 = auxb[1, base : base + cnt].astype(NP_BF16).astype(np.float64)
            t1 = -2.0 * (nrm[rows].astype(np.float64) + c0 - 0.5 * r1)
            diag_corr += np.sqrt(np.maximum(t1, 0.0)).sum()
        in_maps.append(
            {
                "xgt": np.ascontiguousarray(xgT.astype(NP_FP8)),
                "aux": np.ascontiguousarray(auxb.astype(NP_BF16)),
            }
        )
    return in_maps, diag_corr


def kernel(x, target):
    t = np.asarray(target).astype(np.int64).ravel()
    counts = np.bincount(t, minlength=NCLS)
    assert counts.max() <= SLOT_ROWS, "class larger than a slot"
    ranks, ws = _plan(counts)
    key = tuple(int(w) for w in ws)
    if key not in _prog_cache:
        _prog_cache[key] = _build(ws)
    nc = _prog_cache[key]
    in_maps, diag_corr = _prep_inputs(x, target, ranks, ws)
    global LAST_RESULTS
    results = run_bass_kernel_spmd(nc, in_maps, list(range(NCORES)), trace=TRACE)
    LAST_RESULTS = results
    total = float(
        sum(np.asarray(r["out"], dtype=np.float64).sum() for r in results.results)
    )
    total -= diag_corr
    return np.float32(total / 2.0 / B)


# revision 14
# speedup vs baseline: 3.7762x; 1.0519x over previous
"""BatchCenterLoss Trainium2 kernel (8 NeuronCores, SPMD via bass_utils).

Loss = sum over same-class pairs (i != j) of ||x_i - x_j|| / 2 / B.

Strategy v3 -- class-sharded data-parallel, single-ACT-pass:
Only same-class pairs contribute. The host sorts rows by class, assigns
classes to (core, slot) round-robin by descending count (slot b across all
cores holds classes of similar size), and uploads per core:
  - xgT  [128=D, R] bf16: the core's rows PRE-TRANSPOSED, each slot padded
    to 256 rows (pad cols are zero),
  - rowb [2, R]  bf16: p0 = 1/0 (real/pad), p1 = row norm n_i (0 for pad),
  - colb [2, RW] bf16: per-slot W-wide col windows: p0 = -(0.5 n_j + delta)
    (0 for pad), p1 = -0.5 (0 for pad).
Per row-tile (slot b, half h) the PE produces, in PSUM,
  psg = x_r^T x_c - 0.5 n_r - 0.5 n_c - delta          (= -0.5 sqdist - d)
with ONE K=128 gram matmul plus ONE K=2 rank-1 matmul (cost on PE is
K-independent). Pad rows/cols come out EXACTLY 0 (zero x cols and zero
rank-1 multipliers), so no Relu pass and no diagonal mask are needed: the
ACT engine does a single in-place Sqrt(scale=-2) with accum_out row-sums
over a 4-bank strided PSUM access pattern. delta keeps the diagonal's fp
noise inside Sqrt's valid domain [0, 2^118]; the host subtracts the
predicted diagonal contribution (known to fp32 accumulation-order noise)
and scales by 1/(2B).

Engine budget per core (cost model): ACT ~5.7us (bottleneck), PE ~4-7us,
DMA ~3us, DVE/Pool 0. Baseline v1 was ACT ~26us + Pool ~29us (indirect
DMA descriptor generation).

Hardware notes (learned the hard way; sim does NOT catch these):
  - build on bacc.Bacc and call nc.compile() -- it splits multi-semaphore
    waits that walrus's LDWEIGHTS lowering cannot encode.
  - engines cannot address SBUF starting at partition 1 (only 0/32/64/96).
"""

from contextlib import ExitStack

import numpy as np
import ml_dtypes

import concourse.bass as bass
import concourse.tile as tile
from concourse import bacc, mybir
from concourse.bass_utils import run_bass_kernel_spmd

B = 16384
D = 128
NCLS = 100
NCORES = 8
NSLOT = 13
SLOT_ROWS = 256
R = NSLOT * SLOT_ROWS
# ACT op groups: each group owns one PSUM tile (PSUM deps are tile-granular,
# so a tile is written by exactly the slots of its group, then read once by
# ACT). tags map to bank counts; bufs controls ring depth. Sum of
# banks*bufs over tags must be <= 8.
OPGROUPS = ((0,), (1, 2, 3), (4, 5, 6, 7), (8, 9, 10), (11, 12))
GROUPTAGS = ("A", "B", "C", "B", "C")
TAGBANKS = {"A": 1, "B": 3, "C": 4}
TAGBUFS = {"A": 1, "B": 1, "C": 1}
DELTA = 0.5
NWARM = 16  # PE pstate warmup matmuls (first ~11 PE instrs run below peak)
# xgt DMA chunks (slot ranges); issue order: chunk0, aux, remaining chunks.
XCHUNKS = ((0, 4), (4, 8), (8, 13))

F32 = mybir.dt.float32
BF16 = mybir.dt.bfloat16
FP8 = mybir.dt.float8e4
NP_BF16 = np.dtype(ml_dtypes.bfloat16)
NP_FP8 = np.dtype(mybir.dt.np(FP8))

_prog_cache = {}
TRACE = False
LAST_RESULTS = None


def _build(ws):
    """ws: per-slot col-window widths (uniform within each ACT group)."""
    ng = len(OPGROUPS)
    offs = np.concatenate([[0], np.cumsum(ws)]).astype(int)
    RW = int(offs[-1])

    nc = bacc.Bacc("TRN2", target_bir_lowering=False, debug=False)
    xgt = nc.dram_tensor("xgt", [128, R], FP8, kind="ExternalInput").ap()
    aux = nc.dram_tensor("aux", [2, R + RW], BF16, kind="ExternalInput").ap()
    out = nc.dram_tensor("out", [128, ng], F32, kind="ExternalOutput").ap()

    with ExitStack() as ctx:
        tc = ctx.enter_context(tile.TileContext(nc))
        const = ctx.enter_context(tc.tile_pool(name="const", bufs=1))
        psp = ctx.enter_context(tc.tile_pool(name="ps", bufs=1, space="PSUM"))

        xgt_sb = const.tile([128, R], FP8)
        aux_sb = const.tile([2, R + RW], BF16)
        rs = const.tile([128, ng], F32)
        wsb = const.tile([128, 16], BF16)

        # PE pstate warmup: engine idle until DMA lands; burn the slow-ramp
        # instructions on tiny matmuls whose results get overwritten.
        nc.vector.memset(wsb[:], 1.0)

        g_tiles = []
        for g, tag in enumerate(GROUPTAGS):
            gt = psp.tile(
                [128, TAGBANKS[tag] * 512],
                F32,
                tag=tag,
                bufs=TAGBUFS[tag],
                name=f"grp{g}",
            )
            g_tiles.append(gt)
        for i in range(NWARM):
            nc.tensor.matmul(
                out=g_tiles[0][0:16, 0:16],
                lhsT=wsb[:, 0:16],
                rhs=wsb[:, 0:16],
                start=True,
                stop=True,
            )

        # DMA order: first xgt chunk, then aux, then remaining chunks.
        lo, hi = XCHUNKS[0][0] * SLOT_ROWS, XCHUNKS[0][1] * SLOT_ROWS
        nc.sync.dma_start(out=xgt_sb[:, lo:hi], in_=xgt[:, lo:hi])
        nc.sync.dma_start(out=aux_sb[:], in_=aux)
        for s0, s1 in XCHUNKS[1:]:
            lo, hi = s0 * SLOT_ROWS, s1 * SLOT_ROWS
            nc.sync.dma_start(out=xgt_sb[:, lo:hi], in_=xgt[:, lo:hi])

        for g, slots in enumerate(OPGROUPS):
            W = int(ws[slots[0]])
            grp = g_tiles[g]
            for qi, s in enumerate(slots):
                base = s * SLOT_ROWS
                O = int(offs[s])
                q0 = qi * 512
                for h in (0, 1):
                    dst = grp[:, q0 + h * W : q0 + (h + 1) * W]
                    nc.tensor.matmul(
                        out=dst,
                        lhsT=xgt_sb[:, base + h * 128 : base + h * 128 + 128],
                        rhs=xgt_sb[:, base : base + W],
                        start=True,
                        stop=False,
                    )
                    nc.tensor.matmul(
                        out=dst,
                        lhsT=aux_sb[:, base + h * 128 : base + h * 128 + 128],
                        rhs=aux_sb[:, R + O : R + O + W],
                        start=False,
                        stop=True,
                    )
            if len(slots) > 1:
                ap_in = grp[:, 0 : len(slots) * 512].rearrange(
                    "p (b w) -> p b w", w=512
                )[:, 0 : len(slots), 0 : 2 * W]
            else:
                ap_in = grp[:, 0 : 2 * W]
            nc.scalar.activation(
                out=ap_in,
                in_=ap_in,
                func=mybir.ActivationFunctionType.Sqrt,
                scale=-2.0,
                accum_out=rs[:, g : g + 1],
            )

        nc.sync.dma_start(out=out[:, :], in_=rs[:])

    nc.compile()
    return nc


def _plan(counts):
    """Assign class ranks to (slot, core); slot widths uniform per ACT group."""
    ranks = np.argsort(counts, kind="stable")[::-1]  # class ids, count desc
    ws = np.zeros(NSLOT, dtype=np.int64)
    for slots in OPGROUPS:
        k0 = 8 * slots[0]
        mx = int(counts[ranks[k0]]) if k0 < len(ranks) else 8
        W = min(((mx + 7) // 8) * 8, SLOT_ROWS)
        for s in slots:
            ws[s] = W
    return ranks, ws


def _prep_inputs(x, target, ranks, ws):
    offs = np.concatenate([[0], np.cumsum(ws)]).astype(int)
    RW = int(offs[-1])
    t = np.asarray(target).astype(np.int64).ravel()
    order = np.argsort(t, kind="stable")
    counts = np.bincount(t, minlength=NCLS)
    starts = np.concatenate([[0], np.cumsum(counts)])

    x8 = np.asarray(x, dtype=np.float32).astype(NP_FP8)
    x832 = x8.astype(np.float32)
    nrm = (x832 * x832).sum(axis=1, dtype=np.float32)  # matches PE's fp32 acc

    in_maps = []
    diag_corr = 0.0
    for core in range(NCORES):
        xgT = np.zeros((128, R), dtype=np.float32)
        auxb = np.zeros((2, R + RW), dtype=np.float32)
        for b in range(NSLOT):
            k = 8 * b + core
            if k >= NCLS:
                continue
            cls = int(ranks[k])
            cnt = int(counts[cls])
            rows = order[starts[cls] : starts[cls] + cnt]
            base = b * SLOT_ROWS
            O = int(offs[b])
            xgT[:, base : base + cnt] = x832[rows].T
            auxb[0, base : base + cnt] = 1.0
            auxb[1, base : base + cnt] = nrm[rows]
            auxb[0, R + O : R + O + cnt] = -(0.5 * nrm[rows] + DELTA)
            auxb[1, R + O : R + O + cnt] = -0.5
            # predicted diagonal contribution (device computes bf16-rounded
            # rank-1 terms against the exact fp32 gram diagonal = nrm)
            c0 = auxb[0, R + O : R + O + cnt].astype(NP_BF16).astype(np.float64)
            r1 = auxb[1, base : base + cnt].astype(NP_BF16).astype(np.float64)
            t1 = -2.0 * (nrm[rows].astype(np.float64) + c0 - 0.5 * r1)
            diag_corr += np.sqrt(np.maximum(t1, 0.0)).sum()
        in_maps.append(
            {
                "xgt": np.ascontiguousarray(xgT.astype(NP_FP8)),
                "aux": np.ascontiguousarray(auxb.astype(NP_BF16)),
            }
        )
    return in_maps, diag_corr


def kernel(x, target):
    t = np.asarray(target).astype(np.int64).ravel()
    counts = np.bincount(t, minlength=NCLS)
    assert counts.max() <= SLOT_ROWS, "class larger than a slot"
    ranks, ws = _plan(counts)
    key = tuple(int(w) for w in ws)
    if key not in _prog_cache:
        _prog_cache[key] = _build(ws)
    nc = _prog_cache[key]
    in_maps, diag_corr = _prep_inputs(x, target, ranks, ws)
    global LAST_RESULTS
    results = run_bass_kernel_spmd(nc, in_maps, list(range(NCORES)), trace=TRACE)
    LAST_RESULTS = results
    total = float(
        sum(np.asarray(r["out"], dtype=np.float64).sum() for r in results.results)
    )
    total -= diag_corr
    return np.float32(total / 2.0 / B)
